# revision 1
# baseline (speedup 1.0000x reference)
"""Graph Wavelet NN (2-layer) Trainium2 kernel, 8-core row-parallel.

Math per layer:  out = (wavelets * filt[None,:]) @ (wavelets_inv @ (x @ W))
Sharding: core i owns row-block rows_i (1024 rows).
  - s = wavelets_inv @ t is computed as sum over cores of
    Winv[:, cols_i] @ t[rows_i]  -> AllReduce (one 8MB collective per layer).
  - out rows: (wavelets*filt)[rows_i, :] @ s  (local after AllReduce).
Host pre-transposes the stationary operands:
  winvT = wavelets_inv.T[rows_i, :]          [1024, 8192]
  a1    = (wavelets[rows_i]*f1).T            [8192, 1024]
  a2    = (wavelets[rows_i]*f2).T            [8192, 1024]
  xT    = input.T[:, rows_i]                 [512, 1024]
Big streams (winvT, a1, a2, AR payload) in bf16, small matmuls in float32r;
all accumulation fp32 in PSUM. Verified on HW: rel err 6.3e-3.
"""

import sys

if "/opt/trn_rl_repo" not in sys.path:
    sys.path.insert(0, "/opt/trn_rl_repo")

import numpy as np

import concourse.bass as bass
import concourse.mybir as mybir
import concourse.tile as tile
from concourse import bacc, bass_utils

N = 8192
F = 512
C = 256
NCORES = 8
R = N // NCORES  # 1024 rows per core

F32 = mybir.dt.float32
F32R = mybir.dt.float32r
BF16 = mybir.dt.bfloat16
USE_BF16 = True
BIG = BF16 if USE_BF16 else F32R
import ml_dtypes
NP_BIG = ml_dtypes.bfloat16 if USE_BF16 else np.float32

MBLK = 512                  # P-phase output row block
NMBLK = N // MBLK           # 16
NKC_LOC = R // 128          # 8   local-k chunks (P phases)
NKC_F = F // 128            # 4   k chunks for x @ W1
NKC_GLOB = N // 128         # 64  global-k chunks (h/out phases)
NMT = R // 128              # 8   local row tiles


def r(ap):
    return ap.bitcast(F32R)


def build_kernel(sim_single_core=False):
    nc = bacc.Bacc(
        "TRN2",
        target_bir_lowering=False,
        debug=False,
        num_devices=1 if sim_single_core else NCORES,
    )

    xT = nc.dram_tensor("xT", [F, R], F32, kind="ExternalInput")
    w1 = nc.dram_tensor("w1", [F, C], F32, kind="ExternalInput")
    w2 = nc.dram_tensor("w2", [C, C], F32, kind="ExternalInput")
    winvT = nc.dram_tensor("winvT", [R, N], BIG if USE_BF16 else F32, kind="ExternalInput")
    a1 = nc.dram_tensor("a1", [N, R], BIG if USE_BF16 else F32, kind="ExternalInput")
    a2 = nc.dram_tensor("a2", [N, R], BIG if USE_BF16 else F32, kind="ExternalInput")
    outT = nc.dram_tensor("outT", [C, R], F32, kind="ExternalOutput")

    rg = [list(range(NCORES))]

    with tile.TileContext(nc) as tc:
        with (
            tc.tile_pool(name="dram", bufs=1, space="DRAM") as dram,
            tc.tile_pool(name="const", bufs=1) as const,
            tc.tile_pool(name="winvp", bufs=6) as winvp,
            tc.tile_pool(name="ap_pool", bufs=6) as ap_pool,
            tc.tile_pool(name="stage", bufs=4) as stage,
            tc.tile_pool(name="psum256", bufs=4, space="PSUM") as psum256,
            tc.tile_pool(name="psum1k", bufs=2, space="PSUM") as psum1k,
        ):
            p1_d = dram.tile([N, C], BIG)
            p2_d = dram.tile([N, C], BIG)
            s1_d = dram.tile([N, C], BIG, addr_space="Shared")
            s2_d = dram.tile([N, C], BIG, addr_space="Shared")

            # ---- persistent SBUF ----
            xT_sb = const.tile([128, NKC_F, R], F32R)      # 16KB/part
            w1_sb = const.tile([128, NKC_F, C], F32R)      # 4KB/part
            w2_sb = const.tile([128, 2, C], F32R)          # 2KB/part
            t1_sb = const.tile([128, NKC_LOC, C], BIG)    # 8KB/part
            t2_sb = const.tile([128, NKC_LOC, C], BIG)    # 8KB/part
            s_sb = const.tile([128, NKC_GLOB, C], BIG)    # 64KB/part
            h1T_sb = const.tile([128, 2, R], F32R)         # 8KB/part

            nc.sync.dma_start(
                out=xT_sb[:], in_=xT.ap().rearrange("(kc p) m -> p kc m", p=128).bitcast(F32R)
            )
            nc.sync.dma_start(
                out=w1_sb[:], in_=w1.ap().rearrange("(kc p) n -> p kc n", p=128).bitcast(F32R)
            )
            nc.sync.dma_start(
                out=w2_sb[:], in_=w2.ap().rearrange("(kc p) n -> p kc n", p=128).bitcast(F32R)
            )

            # ---- phase 0: t1 = x @ W1  (local rows) ----
            for mt in range(NMT):
                pt = psum256.tile([128, C], F32, tag="acc256")
                for kc in range(NKC_F):
                    nc.tensor.matmul(
                        pt[:],
                        xT_sb[:, kc, mt * 128:(mt + 1) * 128],
                        w1_sb[:, kc, :],
                        start=(kc == 0),
                        stop=(kc == NKC_F - 1),
                    )
                nc.vector.tensor_copy(t1_sb[:, mt, :], pt[:])

            # ---- P phases: partial = Winv[:, cols_i] @ t ----
            def p_phase(t_sb, p_d):
                for mb in range(NMBLK):
                    wv = winvp.tile([128, NKC_LOC, MBLK], BIG, tag="wv")
                    nc.sync.dma_start(
                        out=wv[:],
                        in_=winvT.ap()[:, mb * MBLK:(mb + 1) * MBLK].rearrange(
                            "(kc p) m -> p kc m", p=128
                        ),
                    )
                    st = stage.tile([128, MBLK // 128, C], BIG, tag="st4")
                    for mt in range(MBLK // 128):
                        pt = psum256.tile([128, C], F32, tag="acc256")
                        for kc in range(NKC_LOC):
                            nc.tensor.matmul(
                                pt[:],
                                wv[:, kc, mt * 128:(mt + 1) * 128],
                                t_sb[:, kc, :],
                                start=(kc == 0),
                                stop=(kc == NKC_LOC - 1),
                            )
                        nc.vector.tensor_copy(st[:, mt, :], pt[:])
                    row0 = mb * MBLK
                    nc.sync.dma_start(
                        out=p_d[row0:row0 + MBLK, :].rearrange(
                            "(mt p) n -> p mt n", p=128
                        ),
                        in_=st[:],
                    )

            # ---- big phases: accT = A.T @ s  ([C, R] in psum) ----
            def big_phase(a_dram, out_cb):
                accs = [
                    psum1k.tile([128, R], F32, tag="acc1k", name=f"acc1k_{j}")
                    for j in range(2)
                ]
                for kci in range(NKC_GLOB // 2):
                    at = ap_pool.tile([128, 2, R], BIG, tag="at")
                    for sub in range(2):
                        kc = kci * 2 + sub
                        nc.sync.dma_start(
                            out=at[:, sub, :],
                            in_=a_dram.ap()[kc * 128:(kc + 1) * 128, :],
                        )
                    for sub in range(2):
                        kc = kci * 2 + sub
                        for nch in range(2):
                            for mh in range(2):
                                nc.tensor.matmul(
                                    accs[nch][:, mh * 512:(mh + 1) * 512],
                                    s_sb[:, kc, nch * 128:(nch + 1) * 128],
                                    at[:, sub, mh * 512:(mh + 1) * 512],
                                    start=(kc == 0),
                                    stop=(kc == NKC_GLOB - 1),
                                )
                for nch in range(2):
                    out_cb(nch, accs[nch])

            def load_s(s_d):
                for q in range(4):
                    kc0 = q * (NKC_GLOB // 4)
                    nc.sync.dma_start(
                        out=s_sb[:, kc0:kc0 + NKC_GLOB // 4, :],
                        in_=s_d[kc0 * 128:(kc0 + NKC_GLOB // 4) * 128, :].rearrange(
                            "(kc p) n -> p kc n", p=128
                        ),
                    )

            def all_reduce(p_d, s_d):
                if sim_single_core:
                    nc.sync.dma_start(out=s_d[:, :], in_=p_d[:, :])
                else:
                    nc.gpsimd.collective_compute(
                        "AllReduce",
                        mybir.AluOpType.add,
                        replica_groups=rg,
                        ins=[p_d.opt()],
                        outs=[s_d.opt()],
                    )

            # ================= layer 1 =================
            p_phase(t1_sb, p1_d)
            all_reduce(p1_d, s1_d)
            load_s(s1_d)

            def relu_out(nch, acc):
                nc.scalar.activation(
                    h1T_sb[:, nch, :], acc[:], mybir.ActivationFunctionType.Relu
                )

            big_phase(a1, relu_out)

            # t2 = h1 @ W2
            for mt in range(NMT):
                pt = psum256.tile([128, C], F32, tag="acc256")
                for kc in range(2):
                    nc.tensor.matmul(
                        pt[:],
                        h1T_sb[:, kc, mt * 128:(mt + 1) * 128],
                        w2_sb[:, kc, :],
                        start=(kc == 0),
                        stop=(kc == 1),
                    )
                nc.vector.tensor_copy(t2_sb[:, mt, :], pt[:])

            # ================= layer 2 =================
            p_phase(t2_sb, p2_d)
            all_reduce(p2_d, s2_d)
            load_s(s2_d)

            def store_out(nch, acc):
                # h1T_sb is dead after the t2 phase; reuse it as staging
                nc.vector.tensor_copy(h1T_sb[:, nch, :], acc[:])
                nc.sync.dma_start(
                    out=outT.ap()[nch * 128:(nch + 1) * 128, :].bitcast(F32R),
                    in_=h1T_sb[:, nch, :],
                )

            big_phase(a2, store_out)

    nc.compile()
    return nc


_NC_CACHE = {}


def _get_nc():
    if "nc" not in _NC_CACHE:
        _NC_CACHE["nc"] = build_kernel()
    return _NC_CACHE["nc"]


def make_in_maps(input, wavelets, wavelets_inv, W1, W2, filter1, filter2):
    input = np.asarray(input, np.float32)
    wavelets = np.asarray(wavelets, np.float32)
    wavelets_inv = np.asarray(wavelets_inv, np.float32)
    W1 = np.ascontiguousarray(np.asarray(W1, np.float32))
    W2 = np.ascontiguousarray(np.asarray(W2, np.float32))
    filter1 = np.asarray(filter1, np.float32)
    filter2 = np.asarray(filter2, np.float32)

    xTf = np.ascontiguousarray(input.T)          # [F, N]
    in_maps = []
    for i in range(NCORES):
        r0, r1 = i * R, (i + 1) * R
        in_maps.append(
            {
                "xT": np.ascontiguousarray(xTf[:, r0:r1]),
                "w1": W1,
                "w2": W2,
                "winvT": np.ascontiguousarray(wavelets_inv[:, r0:r1].T).astype(NP_BIG),
                "a1": np.ascontiguousarray((wavelets[r0:r1] * filter1).T).astype(NP_BIG),
                "a2": np.ascontiguousarray((wavelets[r0:r1] * filter2).T).astype(NP_BIG),
            }
        )
    return in_maps


def run(in_maps, trace=False, **kw):
    nc = _get_nc()
    return bass_utils.run_bass_kernel_spmd(
        nc, in_maps, core_ids=list(range(NCORES)), trace=trace, **kw
    )


def kernel(input, wavelets, wavelets_inv, W1, W2, filter1, filter2):
    in_maps = make_in_maps(
        input, wavelets, wavelets_inv, W1, W2, filter1, filter2
    )
    res = run(in_maps)
    out = np.empty((N, C), np.float32)
    for i in range(NCORES):
        out[i * R:(i + 1) * R, :] = res.results[i]["outT"].T
    return out



# revision 10
# speedup vs baseline: 1.0180x; 1.0180x over previous
"""Graph Wavelet NN (2-layer) Trainium2 kernel, 8-core row-parallel.

Math per layer:  out = (wavelets * filt[None,:]) @ (wavelets_inv @ (x @ W))
Sharding: core i owns row-block rows_i (1024 rows).
  - s = wavelets_inv @ t is computed as sum over cores of
    Winv[:, cols_i] @ t[rows_i]  -> AllReduce (one 8MB collective per layer).
  - out rows: (wavelets*filt)[rows_i, :] @ s  (local after AllReduce).
Host pre-transposes the stationary operands:
  winvT = wavelets_inv.T[rows_i, :]          [1024, 8192]
  a1    = (wavelets[rows_i]*f1).T            [8192, 1024]
  a2    = (wavelets[rows_i]*f2).T            [8192, 1024]
  xT    = input.T[:, rows_i]                 [512, 1024]
Big streams (winvT, a1, a2, AR payload) in bf16, small matmuls in float32r;
all accumulation fp32 in PSUM. Verified on HW: rel err 6.3e-3.
"""

import sys

if "/opt/trn_rl_repo" not in sys.path:
    sys.path.insert(0, "/opt/trn_rl_repo")

import numpy as np

import concourse.bass as bass
import concourse.mybir as mybir
import concourse.tile as tile
from concourse import bacc, bass_utils

N = 8192
F = 512
C = 256
NCORES = 8
R = N // NCORES  # 1024 rows per core

F32 = mybir.dt.float32
F32R = mybir.dt.float32r
BF16 = mybir.dt.bfloat16
USE_BF16 = True
BIG = BF16 if USE_BF16 else F32R
import ml_dtypes
NP_BIG = ml_dtypes.bfloat16 if USE_BF16 else np.float32

MBLK = 512                  # P-phase output row block
NMBLK = N // MBLK           # 16
NAR = 4                     # AllReduce chunks per layer
MB_PER_AR = NMBLK // NAR    # 4 MBLKs per AR chunk
AR_ROWS = N // NAR          # 2048 rows per AR chunk
NKC_LOC = R // 128          # 8   local-k chunks (P phases)
NKC_F = F // 128            # 4   k chunks for x @ W1
NKC_GLOB = N // 128         # 64  global-k chunks (h/out phases)
NMT = R // 128              # 8   local row tiles


def r(ap):
    return ap.bitcast(F32R)


def build_kernel(sim_single_core=False):
    nc = bacc.Bacc(
        "TRN2",
        target_bir_lowering=False,
        debug=False,
        num_devices=1 if sim_single_core else NCORES,
    )

    xT = nc.dram_tensor("xT", [F, R], F32, kind="ExternalInput")
    w1 = nc.dram_tensor("w1", [F, C], F32, kind="ExternalInput")
    w2 = nc.dram_tensor("w2", [C, C], F32, kind="ExternalInput")
    winvT = nc.dram_tensor("winvT", [R, N], BIG if USE_BF16 else F32, kind="ExternalInput")
    a1 = nc.dram_tensor("a1", [N, R], BIG if USE_BF16 else F32, kind="ExternalInput")
    a2 = nc.dram_tensor("a2", [N, R], BIG if USE_BF16 else F32, kind="ExternalInput")
    outT = nc.dram_tensor("outT", [C, R], F32, kind="ExternalOutput")

    rg = [list(range(NCORES))]

    with tile.TileContext(nc) as tc:
        with (
            tc.tile_pool(name="dram", bufs=1, space="DRAM") as dram,
            tc.tile_pool(name="const", bufs=1) as const,
            tc.tile_pool(name="winvp", bufs=6) as winvp,
            tc.tile_pool(name="ap_pool", bufs=6) as ap_pool,
            tc.tile_pool(name="stage", bufs=4) as stage,
            tc.tile_pool(name="psum256", bufs=4, space="PSUM") as psum256,
            tc.tile_pool(name="psum1k", bufs=2, space="PSUM") as psum1k,
        ):
            p1_d = dram.tile([N, C], BIG)
            p2_d = dram.tile([N, C], BIG)
            # one Shared tile per AR chunk (Shared DRAM allows one writer)
            s1_ds = [
                dram.tile([AR_ROWS, C], BIG, addr_space="Shared", name=f"s1c{g}")
                for g in range(NAR)
            ]
            s2_ds = [
                dram.tile([AR_ROWS, C], BIG, addr_space="Shared", name=f"s2c{g}")
                for g in range(NAR)
            ]

            # ---- persistent SBUF ----
            xT_sb = const.tile([128, NKC_F, R], F32R)      # 16KB/part
            w1_sb = const.tile([128, NKC_F, C], F32R)      # 4KB/part
            w2_sb = const.tile([128, 2, C], F32R)          # 2KB/part
            t1_sb = const.tile([128, NKC_LOC, C], BIG)    # 8KB/part
            t2_sb = const.tile([128, NKC_LOC, C], BIG)    # 8KB/part
            s_sb = const.tile([128, NKC_GLOB, C], BIG)    # 64KB/part
            h1T_sb = const.tile([128, 2, R], F32R)         # 8KB/part

            nc.sync.dma_start(
                out=xT_sb[:], in_=xT.ap().rearrange("(kc p) m -> p kc m", p=128).bitcast(F32R)
            )
            nc.sync.dma_start(
                out=w1_sb[:], in_=w1.ap().rearrange("(kc p) n -> p kc n", p=128).bitcast(F32R)
            )
            nc.sync.dma_start(
                out=w2_sb[:], in_=w2.ap().rearrange("(kc p) n -> p kc n", p=128).bitcast(F32R)
            )

            # ---- phase 0: t1 = x @ W1  (local rows) ----
            for mt in range(NMT):
                pt = psum256.tile([128, C], F32, tag="acc256")
                for kc in range(NKC_F):
                    nc.tensor.matmul(
                        pt[:],
                        xT_sb[:, kc, mt * 128:(mt + 1) * 128],
                        w1_sb[:, kc, :],
                        start=(kc == 0),
                        stop=(kc == NKC_F - 1),
                    )
                nc.vector.tensor_copy(t1_sb[:, mt, :], pt[:])

            # ---- P phases: partial = Winv[:, cols_i] @ t ----
            # Chunked: after each group of MBLKs, issue the AllReduce for
            # that row-chunk of the partial product and the SBUF load of
            # the reduced chunk. The collectives run on separate silicon
            # (SDMA + CCE), so they overlap the remaining P-phase matmuls
            # and the big-phase consumption of earlier chunks.
            def p_phase(t_sb, p_d, s_ds):
                for g in range(NAR):
                    for mb in range(g * MB_PER_AR, (g + 1) * MB_PER_AR):
                        wv = winvp.tile([128, NKC_LOC, MBLK], BIG, tag="wv")
                        nc.sync.dma_start(
                            out=wv[:],
                            in_=winvT.ap()[:, mb * MBLK:(mb + 1) * MBLK].rearrange(
                                "(kc p) m -> p kc m", p=128
                            ),
                        )
                        st = stage.tile([128, MBLK // 128, C], BIG, tag="st4")
                        for mt in range(MBLK // 128):
                            pt = psum256.tile([128, C], F32, tag="acc256")
                            for kc in range(NKC_LOC):
                                nc.tensor.matmul(
                                    pt[:],
                                    wv[:, kc, mt * 128:(mt + 1) * 128],
                                    t_sb[:, kc, :],
                                    start=(kc == 0),
                                    stop=(kc == NKC_LOC - 1),
                                )
                            nc.vector.tensor_copy(st[:, mt, :], pt[:])
                        row0 = mb * MBLK
                        nc.sync.dma_start(
                            out=p_d[row0:row0 + MBLK, :].rearrange(
                                "(mt p) n -> p mt n", p=128
                            ),
                            in_=st[:],
                        )
                    r0, r1 = g * AR_ROWS, (g + 1) * AR_ROWS
                    if sim_single_core:
                        nc.sync.dma_start(out=s_ds[g][:, :], in_=p_d[r0:r1, :])
                    else:
                        nc.gpsimd.collective_compute(
                            "AllReduce",
                            mybir.AluOpType.add,
                            replica_groups=rg,
                            ins=[p_d[r0:r1, :].opt()],
                            outs=[s_ds[g][:, :].opt()],
                        )
                    kc0 = g * (NKC_GLOB // NAR)
                    nc.sync.dma_start(
                        out=s_sb[:, kc0:kc0 + NKC_GLOB // NAR, :],
                        in_=s_ds[g][:, :].rearrange("(kc p) n -> p kc n", p=128),
                    )

            # ---- big phases: accT = A.T @ s  ([C, R] in psum) ----
            def big_phase(a_dram, out_cb):
                accs = [
                    psum1k.tile([128, R], F32, tag="acc1k", name=f"acc1k_{j}")
                    for j in range(2)
                ]
                for kci in range(NKC_GLOB // 2):
                    at = ap_pool.tile([128, 2, R], BIG, tag="at")
                    for sub in range(2):
                        kc = kci * 2 + sub
                        nc.sync.dma_start(
                            out=at[:, sub, :],
                            in_=a_dram.ap()[kc * 128:(kc + 1) * 128, :],
                        )
                    for sub in range(2):
                        kc = kci * 2 + sub
                        for nch in range(2):
                            for mh in range(2):
                                nc.tensor.matmul(
                                    accs[nch][:, mh * 512:(mh + 1) * 512],
                                    s_sb[:, kc, nch * 128:(nch + 1) * 128],
                                    at[:, sub, mh * 512:(mh + 1) * 512],
                                    start=(kc == 0),
                                    stop=(kc == NKC_GLOB - 1),
                                )
                for nch in range(2):
                    out_cb(nch, accs[nch])

            # ================= layer 1 =================
            p_phase(t1_sb, p1_d, s1_ds)

            def relu_out(nch, acc):
                nc.scalar.activation(
                    h1T_sb[:, nch, :], acc[:], mybir.ActivationFunctionType.Relu
                )

            big_phase(a1, relu_out)

            # t2 = h1 @ W2
            for mt in range(NMT):
                pt = psum256.tile([128, C], F32, tag="acc256")
                for kc in range(2):
                    nc.tensor.matmul(
                        pt[:],
                        h1T_sb[:, kc, mt * 128:(mt + 1) * 128],
                        w2_sb[:, kc, :],
                        start=(kc == 0),
                        stop=(kc == 1),
                    )
                nc.vector.tensor_copy(t2_sb[:, mt, :], pt[:])

            # ================= layer 2 =================
            p_phase(t2_sb, p2_d, s2_ds)

            def store_out(nch, acc):
                # h1T_sb is dead after the t2 phase; reuse it as staging
                nc.vector.tensor_copy(h1T_sb[:, nch, :], acc[:])
                nc.sync.dma_start(
                    out=outT.ap()[nch * 128:(nch + 1) * 128, :].bitcast(F32R),
                    in_=h1T_sb[:, nch, :],
                )

            big_phase(a2, store_out)

    nc.compile()
    return nc


_NC_CACHE = {}


def _get_nc():
    if "nc" not in _NC_CACHE:
        _NC_CACHE["nc"] = build_kernel()
    return _NC_CACHE["nc"]


def make_in_maps(input, wavelets, wavelets_inv, W1, W2, filter1, filter2):
    input = np.asarray(input, np.float32)
    wavelets = np.asarray(wavelets, np.float32)
    wavelets_inv = np.asarray(wavelets_inv, np.float32)
    W1 = np.ascontiguousarray(np.asarray(W1, np.float32))
    W2 = np.ascontiguousarray(np.asarray(W2, np.float32))
    filter1 = np.asarray(filter1, np.float32)
    filter2 = np.asarray(filter2, np.float32)

    xTf = np.ascontiguousarray(input.T)          # [F, N]
    in_maps = []
    for i in range(NCORES):
        r0, r1 = i * R, (i + 1) * R
        in_maps.append(
            {
                "xT": np.ascontiguousarray(xTf[:, r0:r1]),
                "w1": W1,
                "w2": W2,
                "winvT": np.ascontiguousarray(wavelets_inv[:, r0:r1].T).astype(NP_BIG),
                "a1": np.ascontiguousarray((wavelets[r0:r1] * filter1).T).astype(NP_BIG),
                "a2": np.ascontiguousarray((wavelets[r0:r1] * filter2).T).astype(NP_BIG),
            }
        )
    return in_maps


def run(in_maps, trace=False, **kw):
    nc = _get_nc()
    return bass_utils.run_bass_kernel_spmd(
        nc, in_maps, core_ids=list(range(NCORES)), trace=trace, **kw
    )


def kernel(input, wavelets, wavelets_inv, W1, W2, filter1, filter2):
    in_maps = make_in_maps(
        input, wavelets, wavelets_inv, W1, W2, filter1, filter2
    )
    res = run(in_maps)
    out = np.empty((N, C), np.float32)
    for i in range(NCORES):
        out[i * R:(i + 1) * R, :] = res.results[i]["outT"].T
    return out

